# revision 28
# baseline (speedup 1.0000x reference)
"""Trainium2 Bass kernel for nn_ExoVariateEmbeddingMamba.

Self-contained: accepts FULL (unsharded) inputs, shards the B*V=256 variate
sequences data-parallel across 8 NeuronCores (32 seqs/core), runs a Bass/Tile
kernel via run_bass_kernel_spmd, gathers the full [4, 64, 512] output.

Algorithm per sequence (validated vs reference in numpy):
  x_exo column xv[L]; all pre-SSM projections are rank<=4 in
  (xv, ones, shift(xv), (l>0)) -> done as K<=4 PE matmuls:
    xc_act^T = Silu(a1 (x) xv + cb (x) ones + a0 (x) shift(xv) + c0 (x) i0)
    G^T      = Silu(w1z (x) xv + b1z (x) ones)
    dblr^T   = xprojT[:, :8].T @ xc_act^T          (PE, K=128)
    dt^T     = softplus(dt_wT.T @ dblr^T + dt_b)   (Exp then Ln(1+x) on ACT)
    dtx^T    = dt * xc_act                          (DVE)
  SSM scan per state channel s (64):
    dA   = Exp(A[:, s] * dt)                        (ACT, fused via AP scale)
    Bbc  = broadcast of B[t, s] over partitions     (PE: stride-0 lhsT column)
    u    = dtx * Bbc                                (DVE)
    h    = scan(dA, u)                              (DVE tensor_tensor_scan)
    Cbc  = broadcast of C[t, s]                     (PE)
    GC   = G * Cbc                                  (DVE)
    P_s  = sum_t (h/L) * GC                         (DVE STT accum_out)
  out_row = (sum_s P_s + D * sum_t xc*G/L) @ W2.T + out_b   (PE)
where W2 = out_w @ m_out_w is folded on the host (exact linear-algebra fold).
"""
import numpy as np

import concourse.tile as tile
from concourse import bacc, mybir
from concourse.bass_utils import run_bass_kernel_spmd

# problem shapes (hardcoded per contract)
B, L, V = 4, 1024, 64
D, S, R, DOUT = 128, 64, 8, 512
NCORES = 8
NSEQ = B * V // NCORES  # 32 sequences per core

# consts tile column layout
AL = 0                # A [128, 64]
XP = AL + S           # xprojT [128, 136]
DTW = XP + R + 2 * S  # dt_wT in rows 0..7, 128 cols
LXC = DTW + D         # lhsT_xc rows 0..3 (a1, cb, a0, c0), 128 cols
LZ = LXC + D          # lhsT_z rows 0..1 (w1z, b1z), 128 cols
W2L = LZ + D          # W2T [128, 512]
DTB = W2L + DOUT      # dt_b col
DCL = DTB + 1         # D col
OBL = DCL + 1         # out_b_rep rows 0..31, 512 cols
NCOL = OBL + DOUT

_CACHE = {}
PROFILE = False       # set True to capture NTFF trace / exec time
LAST_EXEC_NS = None   # exec_time_ns of the last run (if PROFILE)
LAST_RESULTS = None
SIM_COMPAT = False    # CoreSim lacks Silu: emulate via Sigmoid + mult
FAST = True           # bf16 broadcasts + GpSimd offload (False = fp32 baseline)

F32 = mybir.dt.float32
AF = mybir.ActivationFunctionType
OP = mybir.AluOpType


def _build(nseq: int):
    nc = bacc.Bacc("TRN2", target_bir_lowering=False, debug=False)
    consts_d = nc.dram_tensor("consts", [D, NCOL], F32, kind="ExternalInput")
    constsb_d = nc.dram_tensor("constsb", [D, 2 * S], mybir.dt.bfloat16,
                               kind="ExternalInput")
    xv4_d = nc.dram_tensor("xv4", [nseq, 4, L], F32, kind="ExternalInput")
    out_d = nc.dram_tensor("out", [nseq, DOUT], F32, kind="ExternalOutput")
    BF = mybir.dt.bfloat16

    HALF = L // 2  # matmul free-dim chunk (<=512)

    with tile.TileContext(nc) as tc:
        with (
            tc.tile_pool(name="const", bufs=1) as constp,
            tc.tile_pool(name="rhs", bufs=3) as rhsp,
            tc.tile_pool(name="seq", bufs=2) as seqp,
            tc.tile_pool(name="inner", bufs=3) as innerp,
            tc.tile_pool(name="acc", bufs=2) as accp,
            tc.tile_pool(name="pmat", bufs=1) as pmatp,
            tc.tile_pool(name="ps_pre", bufs=1, space="PSUM") as ps_pre,
            tc.tile_pool(name="ps_b", bufs=1, space="PSUM") as ps_b,
            tc.tile_pool(name="ps_c", bufs=2, space="PSUM") as ps_c,
        ):
            consts = constp.tile([D, NCOL], F32)
            nc.sync.dma_start(out=consts, in_=consts_d.ap())
            constsb = constp.tile([D, 2 * S], BF)
            nc.sync.dma_start(out=constsb, in_=constsb_d.ap())
            P_mat = pmatp.tile([D, nseq], F32)

            for n in range(nseq):
                rhs4 = rhsp.tile([4, L], F32, tag="rhs4")
                nc.sync.dma_start(out=rhs4, in_=xv4_d.ap()[n])

                # xc_conv^T = lhsT_xc.T @ rhs4  -> Silu -> xc_act [D, L]
                psum_xc = ps_pre.tile([D, L], F32, tag="pre")
                for c in range(2):
                    nc.tensor.matmul(
                        psum_xc[:, c * HALF:(c + 1) * HALF],
                        consts[0:4, LXC:LXC + D],
                        rhs4[0:4, c * HALF:(c + 1) * HALF],
                        start=True, stop=True,
                    )
                xc = seqp.tile([D, L], F32, tag="xc")
                if SIM_COMPAT:
                    nc.scalar.activation(out=xc, in_=psum_xc, func=AF.Sigmoid)
                    nc.vector.tensor_tensor(out=xc, in0=xc, in1=psum_xc,
                                            op=OP.mult)
                else:
                    nc.scalar.activation(out=xc, in_=psum_xc, func=AF.Silu)

                # z^T -> Silu -> G [D, L]
                psum_z = ps_pre.tile([D, L], F32, tag="pre")
                for c in range(2):
                    nc.tensor.matmul(
                        psum_z[:, c * HALF:(c + 1) * HALF],
                        consts[0:2, LZ:LZ + D],
                        rhs4[0:2, c * HALF:(c + 1) * HALF],
                        start=True, stop=True,
                    )
                G = seqp.tile([D, L], F32, tag="G")
                if SIM_COMPAT:
                    nc.scalar.activation(out=G, in_=psum_z, func=AF.Sigmoid)
                    nc.vector.tensor_tensor(out=G, in0=G, in1=psum_z,
                                            op=OP.mult)
                else:
                    nc.scalar.activation(out=G, in_=psum_z, func=AF.Silu)

                # dblr^T [8, L] = xprojT[:, :8].T @ xc
                psum_dtr = ps_pre.tile([R, L], F32, tag="pre")
                for c in range(2):
                    nc.tensor.matmul(
                        psum_dtr[:, c * HALF:(c + 1) * HALF],
                        consts[:, XP:XP + R],
                        xc[:, c * HALF:(c + 1) * HALF],
                        start=True, stop=True,
                    )
                dtr = rhsp.tile([R, L], F32, tag="dtr_sb")
                nc.scalar.copy(out=dtr, in_=psum_dtr)

                # dt_raw^T [D, L] = dt_wT.T @ dblr^T ; dt = softplus(+dt_b)
                psum_dt = ps_pre.tile([D, L], F32, tag="pre")
                for c in range(2):
                    nc.tensor.matmul(
                        psum_dt[:, c * HALF:(c + 1) * HALF],
                        consts[0:R, DTW:DTW + D],
                        dtr[0:R, c * HALF:(c + 1) * HALF],
                        start=True, stop=True,
                    )
                edt = seqp.tile([D, L], F32, tag="edt")
                nc.scalar.activation(out=edt, in_=psum_dt, func=AF.Exp,
                                     bias=consts[:, DTB:DTB + 1])
                dt = seqp.tile([D, L], F32, tag="dt")
                nc.scalar.activation(out=dt, in_=edt, func=AF.Ln, bias=1.0)

                # dtx = dt * xc
                dtx = seqp.tile([D, L], F32, tag="dtx")
                nc.vector.tensor_tensor(out=dtx, in0=dt, in1=xc, op=OP.mult)

                # per-n accumulator: cols 0..63 = s terms, 64 = D-term, 65 = scratch
                P_all = accp.tile([D, S + 2], F32, tag="pall")
                junk = seqp.tile([D, L], F32, tag="junk")

                # D-term: raw (xc/L)*G accum -> col 65; * D -> col 64
                nc.vector.scalar_tensor_tensor(
                    out=junk, in0=xc, scalar=1.0 / L, in1=G,
                    op0=OP.mult, op1=OP.mult,
                    accum_out=P_all[:, S + 1:S + 2])
                nc.vector.tensor_scalar(
                    out=P_all[:, S:S + 1], in0=P_all[:, S + 1:S + 2],
                    scalar1=consts[:, DCL:DCL + 1], scalar2=None, op0=OP.mult)

                if FAST:
                    # bf16 staging copies (cheap 2x DVE copies, once per n)
                    xc_bf = seqp.tile([D, L], BF, tag="xcbf")
                    nc.vector.tensor_copy(out=xc_bf, in_=xc)
                    dtx_bf = seqp.tile([D, L], BF, tag="dtxbf")
                    nc.vector.tensor_copy(out=dtx_bf, in_=dtx)
                    G_bf = seqp.tile([D, L], BF, tag="gbf")
                    nc.vector.tensor_copy(out=G_bf, in_=G)

                pending = None  # software-pipelined STT: (hg, pc, s) of prev iter

                def flush_pending():
                    nonlocal pending
                    if pending is not None:
                        hg_p, pc_p, s_p = pending
                        nc.vector.scalar_tensor_tensor(
                            out=junk, in0=hg_p, scalar=1.0 / L, in1=pc_p,
                            op0=OP.mult, op1=OP.mult,
                            accum_out=P_all[:, s_p:s_p + 1])
                        pending = None

                for s in range(S):
                    # dA = exp(A[:, s] * dt)  (fp32: decay factors must be
                    # accurate — bf16 log-decay error compounds over time)
                    dA = innerp.tile([D, L], F32, tag="dA")
                    nc.scalar.activation(out=dA, in_=dt, func=AF.Exp,
                                         scale=consts[:, AL + s:AL + s + 1])
                    if FAST:
                        # B broadcast in bf16 on PE
                        pb = ps_b.tile([D, L], F32, tag="bb")
                        lhsb = constsb[:, s:s + 1].to_broadcast([D, D])
                        for c in range(2):
                            nc.tensor.matmul(
                                pb[:, c * HALF:(c + 1) * HALF], lhsb,
                                xc_bf[:, c * HALF:(c + 1) * HALF],
                                start=True, stop=True)
                        # stage B to SBUF bf16 on the (slack) Scalar engine
                        b_sb = innerp.tile([D, L], BF, tag="bsb")
                        nc.scalar.copy(out=b_sb, in_=pb)
                        # u = dtx * B  (bf16 ins, fp32 out for the scan)
                        u = innerp.tile([D, L], F32, tag="u")
                        nc.vector.tensor_tensor(out=u, in0=dtx_bf, in1=b_sb,
                                                op=OP.mult)
                        # h = scan(dA fp32, u fp32) -> h bf16
                        h = innerp.tile([D, L], BF, tag="h")
                        nc.vector.tensor_tensor_scan(
                            out=h, data0=dA, data1=u, initial=0.0,
                            op0=OP.mult, op1=OP.add)
                        # hG = h * G on GpSimd (bf16 out: halves SBUF traffic)
                        hg = innerp.tile([D, L], BF, tag="hg")
                        nc.gpsimd.tensor_tensor(out=hg, in0=h, in1=G_bf,
                                                op=OP.mult)
                        # C broadcast in bf16 on PE
                        pc = ps_c.tile([D, L], F32, tag="cb")
                        lhsc = constsb[:, S + s:S + s + 1].to_broadcast([D, D])
                        for c in range(2):
                            nc.tensor.matmul(
                                pc[:, c * HALF:(c + 1) * HALF], lhsc,
                                xc_bf[:, c * HALF:(c + 1) * HALF],
                                start=True, stop=True)
                        # P_s = sum_t (hg/L) * C — deferred one iteration so
                        # the DVE has work while GpSimd computes hg
                        flush_pending()
                        pending = (hg, pc, s)
                        continue

                    # fp32 baseline path
                    pb = ps_b.tile([D, L], F32, tag="bb")
                    lhsb = consts[:, XP + R + s:XP + R + s + 1].to_broadcast([D, D])
                    for c in range(2):
                        nc.tensor.matmul(
                            pb[:, c * HALF:(c + 1) * HALF], lhsb,
                            xc[:, c * HALF:(c + 1) * HALF],
                            start=True, stop=True)
                    u = innerp.tile([D, L], F32, tag="u")
                    nc.vector.tensor_tensor(out=u, in0=dtx, in1=pb, op=OP.mult)
                    # h = scan(dA, u)
                    h = innerp.tile([D, L], F32, tag="h")
                    nc.vector.tensor_tensor_scan(
                        out=h, data0=dA, data1=u, initial=0.0,
                        op0=OP.mult, op1=OP.add)
                    # C broadcast
                    pc = ps_c.tile([D, L], F32, tag="cb")
                    lhsc = consts[:, XP + R + S + s:XP + R + S + s + 1] \
                        .to_broadcast([D, D])
                    for c in range(2):
                        nc.tensor.matmul(
                            pc[:, c * HALF:(c + 1) * HALF], lhsc,
                            xc[:, c * HALF:(c + 1) * HALF],
                            start=True, stop=True)
                    gc = innerp.tile([D, L], F32, tag="gc")
                    nc.vector.tensor_tensor(out=gc, in0=G, in1=pc, op=OP.mult)
                    # P_s = sum_t (h/L) * gc
                    nc.vector.scalar_tensor_tensor(
                        out=junk, in0=h, scalar=1.0 / L, in1=gc,
                        op0=OP.mult, op1=OP.mult,
                        accum_out=P_all[:, s:s + 1])

                flush_pending()
                # P_mat[:, n] = sum over cols 0..64
                nc.vector.tensor_reduce(
                    out=P_mat[:, n:n + 1], in_=P_all[:, 0:S + 1],
                    axis=mybir.AxisListType.X, op=OP.add)

            # out [nseq, DOUT] = P_mat.T @ W2T + out_b
            psum_out = ps_pre.tile([nseq, DOUT], F32, tag="pre")
            nc.tensor.matmul(psum_out, P_mat, consts[:, W2L:W2L + DOUT],
                             start=True, stop=True)
            out_sb = rhsp.tile([nseq, DOUT], F32, tag="outsb")
            nc.vector.tensor_tensor(out=out_sb, in0=psum_out,
                                    in1=consts[0:nseq, OBL:OBL + DOUT],
                                    op=OP.add)
            nc.sync.dma_start(out=out_d.ap(), in_=out_sb)

    nc.compile()
    return nc


def _host_prep(x_exo, in_w, in_b, m_in_w, conv_w, conv_b, xproj_w, dt_w, dt_b,
               A_log, D_in, m_out_w, out_w, out_b):
    f32 = np.float32
    w1 = (m_in_w @ in_w[:, 0]).astype(f32)
    b1 = (m_in_w @ in_b).astype(f32)
    w1x, w1z = w1[:D], w1[D:]
    b1x, b1z = b1[:D], b1[D:]
    cw0 = conv_w[:, 0, 0]
    cw1 = conv_w[:, 0, 1]
    a0 = (cw0 * w1x).astype(f32)
    a1 = (cw1 * w1x).astype(f32)
    c0 = (cw0 * b1x).astype(f32)
    cb = (cw1 * b1x + conv_b).astype(f32)
    A = (-np.exp(A_log)).astype(f32)
    W2 = (out_w @ m_out_w).astype(f32)

    consts = np.zeros((D, NCOL), f32)
    consts[:, AL:AL + S] = A
    consts[:, XP:XP + R + 2 * S] = xproj_w.T
    consts[0:R, DTW:DTW + D] = dt_w.T
    consts[0:4, LXC:LXC + D] = np.stack([a1, cb, a0, c0])
    consts[0:2, LZ:LZ + D] = np.stack([w1z, b1z])
    consts[:, W2L:W2L + DOUT] = W2.T
    consts[:, DTB] = dt_b
    consts[:, DCL] = D_in
    consts[0:NSEQ, OBL:OBL + DOUT] = np.tile(out_b, (NSEQ, 1))

    import ml_dtypes
    constsb = np.ascontiguousarray(
        xproj_w.T[:, R:R + 2 * S]).astype(ml_dtypes.bfloat16)

    xv_all = np.ascontiguousarray(
        x_exo.transpose(0, 2, 1).reshape(B * V, L)).astype(f32)
    xv4 = np.zeros((B * V, 4, L), f32)
    xv4[:, 0] = xv_all                    # xv
    xv4[:, 1] = 1.0                       # ones
    xv4[:, 2, 1:] = xv_all[:, :-1]        # shift(xv)
    xv4[:, 3, 1:] = 1.0                   # i0 = (l > 0)
    return consts, constsb, xv4


def kernel(**inputs):
    consts, constsb, xv4 = _host_prep(
        inputs["x_exo"], inputs["in_w"], inputs["in_b"], inputs["m_in_w"],
        inputs["conv_w"], inputs["conv_b"], inputs["xproj_w"], inputs["dt_w"],
        inputs["dt_b"], inputs["A_log"], inputs["D"], inputs["m_out_w"],
        inputs["out_w"], inputs["out_b"])

    global LAST_EXEC_NS, LAST_RESULTS
    if _CACHE.get("nseq") != NSEQ:
        _CACHE["nc"] = _build(NSEQ)
        _CACHE["nseq"] = NSEQ
    nc = _CACHE["nc"]

    in_maps = []
    for c in range(NCORES):
        in_maps.append({
            "consts": consts,
            "constsb": constsb,
            "xv4": np.ascontiguousarray(xv4[c * NSEQ:(c + 1) * NSEQ]),
        })
    res = run_bass_kernel_spmd(nc, in_maps, core_ids=list(range(NCORES)),
                               trace=PROFILE)
    LAST_EXEC_NS = res.exec_time_ns
    LAST_RESULTS = res
    out = np.concatenate([res.results[c]["out"] for c in range(NCORES)], axis=0)
    if out.shape[0] == B * V:
        out = out.reshape(B, V, DOUT)
    return out.astype(np.float32)


# revision 31
# speedup vs baseline: 1.6758x; 1.6758x over previous
"""Trainium2 Bass kernel for nn_ExoVariateEmbeddingMamba.

Self-contained: accepts FULL (unsharded) inputs, shards the B*V=256 variate
sequences data-parallel across 8 NeuronCores (32 seqs/core), runs a Bass/Tile
kernel via run_bass_kernel_spmd, gathers the full [4, 64, 512] output.

Algorithm per sequence (validated vs reference in numpy):
  x_exo column xv[L]; all pre-SSM projections are rank<=4 in
  (xv, ones, shift(xv), (l>0)) -> done as K<=4 PE matmuls:
    xc_act^T = Silu(a1 (x) xv + cb (x) ones + a0 (x) shift(xv) + c0 (x) i0)
    G^T      = Silu(w1z (x) xv + b1z (x) ones)
    dblr^T   = xprojT[:, :8].T @ xc_act^T          (PE, K=128)
    dt^T     = softplus(dt_wT.T @ dblr^T + dt_b)   (Exp then Ln(1+x) on ACT)
    dtx^T    = dt * xc_act                          (DVE)
  SSM scan per state channel s (64):
    dA   = Exp(A[:, s] * dt)                        (ACT, fused via AP scale)
    Bbc  = broadcast of B[t, s] over partitions     (PE: stride-0 lhsT column)
    u    = dtx * Bbc                                (DVE)
    h    = scan(dA, u)                              (DVE tensor_tensor_scan)
    Cbc  = broadcast of C[t, s]                     (PE)
    GC   = G * Cbc                                  (DVE)
    P_s  = sum_t (h/L) * GC                         (DVE STT accum_out)
  out_row = (sum_s P_s + D * sum_t xc*G/L) @ W2.T + out_b   (PE)
where W2 = out_w @ m_out_w is folded on the host (exact linear-algebra fold).
"""
import numpy as np

import concourse.tile as tile
from concourse import bacc, mybir
from concourse.bass_utils import run_bass_kernel_spmd

# problem shapes (hardcoded per contract)
B, L, V = 4, 1024, 64
D, S, R, DOUT = 128, 64, 8, 512
NCORES = 8
NSEQ = B * V // NCORES  # 32 sequences per core

# consts tile column layout
AL = 0                # A [128, 64]
XP = AL + S           # xprojT [128, 136]
DTW = XP + R + 2 * S  # dt_wT in rows 0..7, 128 cols
LXC = DTW + D         # lhsT_xc rows 0..3 (a1, cb, a0, c0), 128 cols
LZ = LXC + D          # lhsT_z rows 0..1 (w1z, b1z), 128 cols
W2L = LZ + D          # W2T [128, 512]
DTB = W2L + DOUT      # dt_b col
DCL = DTB + 1         # D col
OBL = DCL + 1         # out_b_rep rows 0..31, 512 cols
NCOL = OBL + DOUT

_CACHE = {}
PROFILE = False       # set True to capture NTFF trace / exec time
LAST_EXEC_NS = None   # exec_time_ns of the last run (if PROFILE)
LAST_RESULTS = None
SIM_COMPAT = False    # CoreSim lacks Silu: emulate via Sigmoid + mult
FAST = True           # bf16 broadcasts + GpSimd offload (False = fp32 baseline)

F32 = mybir.dt.float32
AF = mybir.ActivationFunctionType
OP = mybir.AluOpType


def _build(nseq: int):
    nc = bacc.Bacc("TRN2", target_bir_lowering=False, debug=False)
    consts_d = nc.dram_tensor("consts", [D, NCOL], F32, kind="ExternalInput")
    constsb_d = nc.dram_tensor("constsb", [D, 2 * S], mybir.dt.bfloat16,
                               kind="ExternalInput")
    xv4_d = nc.dram_tensor("xv4", [nseq, 4, L], F32, kind="ExternalInput")
    out_d = nc.dram_tensor("out", [nseq, DOUT], F32, kind="ExternalOutput")
    BF = mybir.dt.bfloat16

    HALF = L // 2  # matmul free-dim chunk (<=512)

    with tile.TileContext(nc) as tc:
        with (
            tc.tile_pool(name="const", bufs=1) as constp,
            tc.tile_pool(name="rhs", bufs=3) as rhsp,
            tc.tile_pool(name="seq", bufs=2) as seqp,
            tc.tile_pool(name="inner", bufs=3) as innerp,
            tc.tile_pool(name="acc", bufs=2) as accp,
            tc.tile_pool(name="pmat", bufs=1) as pmatp,
            tc.tile_pool(name="ps_big", bufs=2, space="PSUM") as ps_pre,
            tc.tile_pool(name="ps_b", bufs=1, space="PSUM") as ps_b,
            tc.tile_pool(name="ps_c", bufs=1, space="PSUM") as ps_c,
        ):
            consts = constp.tile([D, NCOL], F32)
            nc.sync.dma_start(out=consts, in_=consts_d.ap())
            constsb = constp.tile([D, 2 * S], BF)
            nc.sync.dma_start(out=constsb, in_=constsb_d.ap())
            P_mat = pmatp.tile([D, nseq], F32)

            for n in range(nseq):
                rhs4 = rhsp.tile([4, L], F32, tag="rhs4")
                nc.sync.dma_start(out=rhs4, in_=xv4_d.ap()[n])

                # xc_conv^T = lhsT_xc.T @ rhs4  -> Silu -> xc_act [D, L]
                psum_xc = ps_pre.tile([D, L], F32, tag="big")
                for c in range(2):
                    nc.tensor.matmul(
                        psum_xc[:, c * HALF:(c + 1) * HALF],
                        consts[0:4, LXC:LXC + D],
                        rhs4[0:4, c * HALF:(c + 1) * HALF],
                        start=True, stop=True,
                    )
                xc = seqp.tile([D, L], F32, tag="xc")
                if SIM_COMPAT:
                    nc.scalar.activation(out=xc, in_=psum_xc, func=AF.Sigmoid)
                    nc.vector.tensor_tensor(out=xc, in0=xc, in1=psum_xc,
                                            op=OP.mult)
                else:
                    nc.scalar.activation(out=xc, in_=psum_xc, func=AF.Silu)

                # z^T -> Silu -> G [D, L]
                psum_z = ps_pre.tile([D, L], F32, tag="big")
                for c in range(2):
                    nc.tensor.matmul(
                        psum_z[:, c * HALF:(c + 1) * HALF],
                        consts[0:2, LZ:LZ + D],
                        rhs4[0:2, c * HALF:(c + 1) * HALF],
                        start=True, stop=True,
                    )
                G = seqp.tile([D, L], F32, tag="G")
                if SIM_COMPAT:
                    nc.scalar.activation(out=G, in_=psum_z, func=AF.Sigmoid)
                    nc.vector.tensor_tensor(out=G, in0=G, in1=psum_z,
                                            op=OP.mult)
                else:
                    nc.scalar.activation(out=G, in_=psum_z, func=AF.Silu)

                # dblr^T [8, L] = xprojT[:, :8].T @ xc
                psum_dtr = ps_pre.tile([R, L], F32, tag="big")
                for c in range(2):
                    nc.tensor.matmul(
                        psum_dtr[:, c * HALF:(c + 1) * HALF],
                        consts[:, XP:XP + R],
                        xc[:, c * HALF:(c + 1) * HALF],
                        start=True, stop=True,
                    )
                dtr = rhsp.tile([R, L], F32, tag="dtr_sb")
                nc.scalar.copy(out=dtr, in_=psum_dtr)

                # dt_raw^T [D, L] = dt_wT.T @ dblr^T ; dt = softplus(+dt_b)
                psum_dt = ps_pre.tile([D, L], F32, tag="big")
                for c in range(2):
                    nc.tensor.matmul(
                        psum_dt[:, c * HALF:(c + 1) * HALF],
                        consts[0:R, DTW:DTW + D],
                        dtr[0:R, c * HALF:(c + 1) * HALF],
                        start=True, stop=True,
                    )
                edt = seqp.tile([D, L], F32, tag="edt")
                nc.scalar.activation(out=edt, in_=psum_dt, func=AF.Exp,
                                     bias=consts[:, DTB:DTB + 1])
                dt = seqp.tile([D, L], F32, tag="dt")
                nc.scalar.activation(out=dt, in_=edt, func=AF.Ln, bias=1.0)

                # dtx = dt * xc
                dtx = seqp.tile([D, L], F32, tag="dtx")
                nc.vector.tensor_tensor(out=dtx, in0=dt, in1=xc, op=OP.mult)

                # per-n accumulator: cols 0..63 = s terms, 64 = D-term, 65 = scratch
                P_all = accp.tile([D, S + 2], F32, tag="pall")
                junk = seqp.tile([D, L], F32, tag="junk")

                # D-term: raw (xc/L)*G accum -> col 65; * D -> col 64
                nc.vector.scalar_tensor_tensor(
                    out=junk, in0=xc, scalar=1.0 / L, in1=G,
                    op0=OP.mult, op1=OP.mult,
                    accum_out=P_all[:, S + 1:S + 2])
                nc.vector.tensor_scalar(
                    out=P_all[:, S:S + 1], in0=P_all[:, S + 1:S + 2],
                    scalar1=consts[:, DCL:DCL + 1], scalar2=None, op0=OP.mult)

                if FAST:
                    # bf16 staging copies (cheap 2x DVE copies, once per n)
                    xc_bf = seqp.tile([D, L], BF, tag="xcbf")
                    nc.vector.tensor_copy(out=xc_bf, in_=xc)
                    dtx_bf = seqp.tile([D, L], BF, tag="dtxbf")
                    nc.vector.tensor_copy(out=dtx_bf, in_=dtx)
                    G_bf = seqp.tile([D, L], BF, tag="gbf")
                    nc.vector.tensor_copy(out=G_bf, in_=G)

                pending = None  # software-pipelined STT: (hg, pc, s) of prev iter

                def flush_pending():
                    nonlocal pending
                    if pending is not None:
                        hg_p, pc_p, s_p = pending
                        nc.vector.scalar_tensor_tensor(
                            out=junk, in0=hg_p, scalar=1.0 / L, in1=pc_p,
                            op0=OP.mult, op1=OP.mult,
                            accum_out=P_all[:, s_p:s_p + 1])
                        pending = None

                for s in range(S):
                    # dA = exp(A[:, s] * dt)  (fp32: decay factors must be
                    # accurate — bf16 log-decay error compounds over time)
                    dA = ps_pre.tile([D, L], F32, tag="big")
                    nc.scalar.activation(out=dA, in_=dt, func=AF.Exp,
                                         scale=consts[:, AL + s:AL + s + 1])
                    if FAST:
                        # B broadcast in bf16 on PE
                        pb = ps_b.tile([D, L], F32, tag="bb")
                        lhsb = constsb[:, s:s + 1].to_broadcast([D, D])
                        for c in range(2):
                            nc.tensor.matmul(
                                pb[:, c * HALF:(c + 1) * HALF], lhsb,
                                xc_bf[:, c * HALF:(c + 1) * HALF],
                                start=True, stop=True)
                        # stage B to SBUF bf16 on the (slack) Scalar engine
                        b_sb = innerp.tile([D, L], BF, tag="bsb")
                        nc.scalar.copy(out=b_sb, in_=pb)
                        # u = dtx * B  (bf16 2x TT)
                        u = innerp.tile([D, L], BF, tag="u")
                        nc.vector.tensor_tensor(out=u, in0=dtx_bf, in1=b_sb,
                                                op=OP.mult)
                        # h = scan(dA fp32, u bf16) -> h bf16
                        h = innerp.tile([D, L], BF, tag="h")
                        nc.vector.tensor_tensor_scan(
                            out=h, data0=dA, data1=u, initial=0.0,
                            op0=OP.mult, op1=OP.add)
                        # hG = h * G on GpSimd (bf16 out: halves SBUF traffic)
                        hg = innerp.tile([D, L], BF, tag="hg")
                        nc.gpsimd.tensor_tensor(out=hg, in0=h, in1=G_bf,
                                                op=OP.mult)
                        # C broadcast in bf16 on PE
                        pc = ps_c.tile([D, L], F32, tag="cb")
                        lhsc = constsb[:, S + s:S + s + 1].to_broadcast([D, D])
                        for c in range(2):
                            nc.tensor.matmul(
                                pc[:, c * HALF:(c + 1) * HALF], lhsc,
                                xc_bf[:, c * HALF:(c + 1) * HALF],
                                start=True, stop=True)
                        # P_s = sum_t (hg/L) * C — deferred one iteration so
                        # the DVE has work while GpSimd computes hg
                        flush_pending()
                        pending = (hg, pc, s)
                        continue

                    # fp32 baseline path
                    pb = ps_b.tile([D, L], F32, tag="bb")
                    lhsb = consts[:, XP + R + s:XP + R + s + 1].to_broadcast([D, D])
                    for c in range(2):
                        nc.tensor.matmul(
                            pb[:, c * HALF:(c + 1) * HALF], lhsb,
                            xc[:, c * HALF:(c + 1) * HALF],
                            start=True, stop=True)
                    u = innerp.tile([D, L], F32, tag="u")
                    nc.vector.tensor_tensor(out=u, in0=dtx, in1=pb, op=OP.mult)
                    # h = scan(dA, u)
                    h = innerp.tile([D, L], F32, tag="h")
                    nc.vector.tensor_tensor_scan(
                        out=h, data0=dA, data1=u, initial=0.0,
                        op0=OP.mult, op1=OP.add)
                    # C broadcast
                    pc = ps_c.tile([D, L], F32, tag="cb")
                    lhsc = consts[:, XP + R + S + s:XP + R + S + s + 1] \
                        .to_broadcast([D, D])
                    for c in range(2):
                        nc.tensor.matmul(
                            pc[:, c * HALF:(c + 1) * HALF], lhsc,
                            xc[:, c * HALF:(c + 1) * HALF],
                            start=True, stop=True)
                    gc = innerp.tile([D, L], F32, tag="gc")
                    nc.vector.tensor_tensor(out=gc, in0=G, in1=pc, op=OP.mult)
                    # P_s = sum_t (h/L) * gc
                    nc.vector.scalar_tensor_tensor(
                        out=junk, in0=h, scalar=1.0 / L, in1=gc,
                        op0=OP.mult, op1=OP.mult,
                        accum_out=P_all[:, s:s + 1])

                flush_pending()
                # P_mat[:, n] = sum over cols 0..64
                nc.vector.tensor_reduce(
                    out=P_mat[:, n:n + 1], in_=P_all[:, 0:S + 1],
                    axis=mybir.AxisListType.X, op=OP.add)

            # out [nseq, DOUT] = P_mat.T @ W2T + out_b
            psum_out = ps_pre.tile([nseq, DOUT], F32, tag="big")
            nc.tensor.matmul(psum_out, P_mat, consts[:, W2L:W2L + DOUT],
                             start=True, stop=True)
            out_sb = rhsp.tile([nseq, DOUT], F32, tag="outsb")
            nc.vector.tensor_tensor(out=out_sb, in0=psum_out,
                                    in1=consts[0:nseq, OBL:OBL + DOUT],
                                    op=OP.add)
            nc.sync.dma_start(out=out_d.ap(), in_=out_sb)

    nc.compile()
    return nc


def _host_prep(x_exo, in_w, in_b, m_in_w, conv_w, conv_b, xproj_w, dt_w, dt_b,
               A_log, D_in, m_out_w, out_w, out_b):
    f32 = np.float32
    w1 = (m_in_w @ in_w[:, 0]).astype(f32)
    b1 = (m_in_w @ in_b).astype(f32)
    w1x, w1z = w1[:D], w1[D:]
    b1x, b1z = b1[:D], b1[D:]
    cw0 = conv_w[:, 0, 0]
    cw1 = conv_w[:, 0, 1]
    a0 = (cw0 * w1x).astype(f32)
    a1 = (cw1 * w1x).astype(f32)
    c0 = (cw0 * b1x).astype(f32)
    cb = (cw1 * b1x + conv_b).astype(f32)
    A = (-np.exp(A_log)).astype(f32)
    W2 = (out_w @ m_out_w).astype(f32)

    consts = np.zeros((D, NCOL), f32)
    consts[:, AL:AL + S] = A
    consts[:, XP:XP + R + 2 * S] = xproj_w.T
    consts[0:R, DTW:DTW + D] = dt_w.T
    consts[0:4, LXC:LXC + D] = np.stack([a1, cb, a0, c0])
    consts[0:2, LZ:LZ + D] = np.stack([w1z, b1z])
    consts[:, W2L:W2L + DOUT] = W2.T
    consts[:, DTB] = dt_b
    consts[:, DCL] = D_in
    consts[0:NSEQ, OBL:OBL + DOUT] = np.tile(out_b, (NSEQ, 1))

    import ml_dtypes
    constsb = np.ascontiguousarray(
        xproj_w.T[:, R:R + 2 * S]).astype(ml_dtypes.bfloat16)

    xv_all = np.ascontiguousarray(
        x_exo.transpose(0, 2, 1).reshape(B * V, L)).astype(f32)
    xv4 = np.zeros((B * V, 4, L), f32)
    xv4[:, 0] = xv_all                    # xv
    xv4[:, 1] = 1.0                       # ones
    xv4[:, 2, 1:] = xv_all[:, :-1]        # shift(xv)
    xv4[:, 3, 1:] = 1.0                   # i0 = (l > 0)
    return consts, constsb, xv4


def kernel(**inputs):
    consts, constsb, xv4 = _host_prep(
        inputs["x_exo"], inputs["in_w"], inputs["in_b"], inputs["m_in_w"],
        inputs["conv_w"], inputs["conv_b"], inputs["xproj_w"], inputs["dt_w"],
        inputs["dt_b"], inputs["A_log"], inputs["D"], inputs["m_out_w"],
        inputs["out_w"], inputs["out_b"])

    global LAST_EXEC_NS, LAST_RESULTS
    if _CACHE.get("nseq") != NSEQ:
        _CACHE["nc"] = _build(NSEQ)
        _CACHE["nseq"] = NSEQ
    nc = _CACHE["nc"]

    in_maps = []
    for c in range(NCORES):
        in_maps.append({
            "consts": consts,
            "constsb": constsb,
            "xv4": np.ascontiguousarray(xv4[c * NSEQ:(c + 1) * NSEQ]),
        })
    res = run_bass_kernel_spmd(nc, in_maps, core_ids=list(range(NCORES)),
                               trace=PROFILE)
    LAST_EXEC_NS = res.exec_time_ns
    LAST_RESULTS = res
    out = np.concatenate([res.results[c]["out"] for c in range(NCORES)], axis=0)
    if out.shape[0] == B * V:
        out = out.reshape(B, V, DOUT)
    return out.astype(np.float32)


# revision 32
# speedup vs baseline: 1.7135x; 1.0225x over previous
"""Trainium2 Bass kernel for nn_ExoVariateEmbeddingMamba.

Self-contained: accepts FULL (unsharded) inputs, shards the B*V=256 variate
sequences data-parallel across 8 NeuronCores (32 seqs/core), runs a Bass/Tile
kernel via run_bass_kernel_spmd, gathers the full [4, 64, 512] output.

Algorithm per sequence (validated vs reference in numpy):
  x_exo column xv[L]; all pre-SSM projections are rank<=4 in
  (xv, ones, shift(xv), (l>0)) -> done as K<=4 PE matmuls:
    xc_act^T = Silu(a1 (x) xv + cb (x) ones + a0 (x) shift(xv) + c0 (x) i0)
    G^T      = Silu(w1z (x) xv + b1z (x) ones)
    dblr^T   = xprojT[:, :8].T @ xc_act^T          (PE, K=128)
    dt^T     = softplus(dt_wT.T @ dblr^T + dt_b)   (Exp then Ln(1+x) on ACT)
    dtx^T    = dt * xc_act                          (DVE)
  SSM scan per state channel s (64):
    dA   = Exp(A[:, s] * dt)                        (ACT, fused via AP scale)
    Bbc  = broadcast of B[t, s] over partitions     (PE: stride-0 lhsT column)
    u    = dtx * Bbc                                (DVE)
    h    = scan(dA, u)                              (DVE tensor_tensor_scan)
    Cbc  = broadcast of C[t, s]                     (PE)
    GC   = G * Cbc                                  (DVE)
    P_s  = sum_t (h/L) * GC                         (DVE STT accum_out)
  out_row = (sum_s P_s + D * sum_t xc*G/L) @ W2.T + out_b   (PE)
where W2 = out_w @ m_out_w is folded on the host (exact linear-algebra fold).
"""
import numpy as np

import concourse.tile as tile
from concourse import bacc, mybir
from concourse.bass_utils import run_bass_kernel_spmd

# problem shapes (hardcoded per contract)
B, L, V = 4, 1024, 64
D, S, R, DOUT = 128, 64, 8, 512
NCORES = 8
NSEQ = B * V // NCORES  # 32 sequences per core

# consts tile column layout
AL = 0                # A [128, 64]
XP = AL + S           # xprojT [128, 136]
DTW = XP + R + 2 * S  # dt_wT in rows 0..7, 128 cols
LXC = DTW + D         # lhsT_xc rows 0..3 (a1, cb, a0, c0), 128 cols
LZ = LXC + D          # lhsT_z rows 0..1 (w1z, b1z), 128 cols
W2L = LZ + D          # W2T [128, 512]
DTB = W2L + DOUT      # dt_b col
DCL = DTB + 1         # D col
OBL = DCL + 1         # out_b_rep rows 0..31, 512 cols
NCOL = OBL + DOUT

_CACHE = {}
PROFILE = False       # set True to capture NTFF trace / exec time
LAST_EXEC_NS = None   # exec_time_ns of the last run (if PROFILE)
LAST_RESULTS = None
SIM_COMPAT = False    # CoreSim lacks Silu: emulate via Sigmoid + mult
FAST = True           # bf16 broadcasts + GpSimd offload (False = fp32 baseline)

F32 = mybir.dt.float32
AF = mybir.ActivationFunctionType
OP = mybir.AluOpType


def _build(nseq: int):
    nc = bacc.Bacc("TRN2", target_bir_lowering=False, debug=False)
    consts_d = nc.dram_tensor("consts", [D, NCOL], F32, kind="ExternalInput")
    constsb_d = nc.dram_tensor("constsb", [D, 2 * S], mybir.dt.bfloat16,
                               kind="ExternalInput")
    xv4_d = nc.dram_tensor("xv4", [nseq, 4, L], F32, kind="ExternalInput")
    out_d = nc.dram_tensor("out", [nseq, DOUT], F32, kind="ExternalOutput")
    BF = mybir.dt.bfloat16

    HALF = L // 2  # matmul free-dim chunk (<=512)

    with tile.TileContext(nc) as tc:
        with (
            tc.tile_pool(name="const", bufs=1) as constp,
            tc.tile_pool(name="rhs", bufs=3) as rhsp,
            tc.tile_pool(name="seq", bufs=2) as seqp,
            tc.tile_pool(name="inner", bufs=3) as innerp,
            tc.tile_pool(name="acc", bufs=2) as accp,
            tc.tile_pool(name="pmat", bufs=1) as pmatp,
            tc.tile_pool(name="ps_big", bufs=2, space="PSUM") as ps_pre,
            tc.tile_pool(name="ps_b", bufs=1, space="PSUM") as ps_b,
            tc.tile_pool(name="ps_c", bufs=1, space="PSUM") as ps_c,
        ):
            consts = constp.tile([D, NCOL], F32)
            nc.sync.dma_start(out=consts, in_=consts_d.ap())
            constsb = constp.tile([D, 2 * S], BF)
            nc.sync.dma_start(out=constsb, in_=constsb_d.ap())
            P_mat = pmatp.tile([D, nseq], F32)

            for n in range(nseq):
                rhs4 = rhsp.tile([4, L], F32, tag="rhs4")
                nc.sync.dma_start(out=rhs4, in_=xv4_d.ap()[n])

                # xc_conv^T = lhsT_xc.T @ rhs4  -> Silu -> xc_act [D, L]
                psum_xc = ps_pre.tile([D, L], F32, tag="big")
                for c in range(2):
                    nc.tensor.matmul(
                        psum_xc[:, c * HALF:(c + 1) * HALF],
                        consts[0:4, LXC:LXC + D],
                        rhs4[0:4, c * HALF:(c + 1) * HALF],
                        start=True, stop=True,
                    )
                xc = seqp.tile([D, L], F32, tag="xc")
                if SIM_COMPAT:
                    nc.scalar.activation(out=xc, in_=psum_xc, func=AF.Sigmoid)
                    nc.vector.tensor_tensor(out=xc, in0=xc, in1=psum_xc,
                                            op=OP.mult)
                else:
                    nc.scalar.activation(out=xc, in_=psum_xc, func=AF.Silu)

                # z^T -> Silu -> G [D, L]
                psum_z = ps_pre.tile([D, L], F32, tag="big")
                for c in range(2):
                    nc.tensor.matmul(
                        psum_z[:, c * HALF:(c + 1) * HALF],
                        consts[0:2, LZ:LZ + D],
                        rhs4[0:2, c * HALF:(c + 1) * HALF],
                        start=True, stop=True,
                    )
                G = seqp.tile([D, L], F32, tag="G")
                if SIM_COMPAT:
                    nc.scalar.activation(out=G, in_=psum_z, func=AF.Sigmoid)
                    nc.vector.tensor_tensor(out=G, in0=G, in1=psum_z,
                                            op=OP.mult)
                else:
                    nc.scalar.activation(out=G, in_=psum_z, func=AF.Silu)

                # dblr^T [8, L] = xprojT[:, :8].T @ xc
                psum_dtr = ps_pre.tile([R, L], F32, tag="big")
                for c in range(2):
                    nc.tensor.matmul(
                        psum_dtr[:, c * HALF:(c + 1) * HALF],
                        consts[:, XP:XP + R],
                        xc[:, c * HALF:(c + 1) * HALF],
                        start=True, stop=True,
                    )
                dtr = rhsp.tile([R, L], F32, tag="dtr_sb")
                nc.scalar.copy(out=dtr, in_=psum_dtr)

                # dt_raw^T [D, L] = dt_wT.T @ dblr^T ; dt = softplus(+dt_b)
                psum_dt = ps_pre.tile([D, L], F32, tag="big")
                for c in range(2):
                    nc.tensor.matmul(
                        psum_dt[:, c * HALF:(c + 1) * HALF],
                        consts[0:R, DTW:DTW + D],
                        dtr[0:R, c * HALF:(c + 1) * HALF],
                        start=True, stop=True,
                    )
                edt = seqp.tile([D, L], F32, tag="edt")
                nc.scalar.activation(out=edt, in_=psum_dt, func=AF.Exp,
                                     bias=consts[:, DTB:DTB + 1])
                dt = seqp.tile([D, L], F32, tag="dt")
                nc.scalar.activation(out=dt, in_=edt, func=AF.Ln, bias=1.0)

                # dtx = dt * xc
                dtx = seqp.tile([D, L], F32, tag="dtx")
                nc.vector.tensor_tensor(out=dtx, in0=dt, in1=xc, op=OP.mult)

                # per-n accumulator: cols 0..63 = s terms, 64 = D-term, 65 = scratch
                P_all = accp.tile([D, S + 2], F32, tag="pall")
                junk = seqp.tile([D, L], F32, tag="junk")

                # D-term: raw (xc/L)*G accum -> col 65; * D -> col 64
                nc.vector.scalar_tensor_tensor(
                    out=junk, in0=xc, scalar=1.0 / L, in1=G,
                    op0=OP.mult, op1=OP.mult,
                    accum_out=P_all[:, S + 1:S + 2])
                nc.vector.tensor_scalar(
                    out=P_all[:, S:S + 1], in0=P_all[:, S + 1:S + 2],
                    scalar1=consts[:, DCL:DCL + 1], scalar2=None, op0=OP.mult)

                if FAST:
                    # bf16 staging copies (cheap 2x DVE copies, once per n)
                    xc_bf = seqp.tile([D, L], BF, tag="xcbf")
                    nc.vector.tensor_copy(out=xc_bf, in_=xc)
                    dtx_bf = seqp.tile([D, L], BF, tag="dtxbf")
                    nc.vector.tensor_copy(out=dtx_bf, in_=dtx)
                    G_bf = seqp.tile([D, L], BF, tag="gbf")
                    nc.vector.tensor_copy(out=G_bf, in_=G)

                pending = None  # software-pipelined STT: (hg, pc, s) of prev iter

                def flush_pending():
                    nonlocal pending
                    if pending is not None:
                        hg_p, pc_p, s_p = pending
                        nc.vector.scalar_tensor_tensor(
                            out=junk, in0=hg_p, scalar=1.0 / L, in1=pc_p,
                            op0=OP.mult, op1=OP.mult,
                            accum_out=P_all[:, s_p:s_p + 1])
                        pending = None

                for s in range(S):
                    # dA = exp(A[:, s] * dt)  (fp32: decay factors must be
                    # accurate — bf16 log-decay error compounds over time)
                    dA = ps_pre.tile([D, L], F32, tag="big")
                    nc.scalar.activation(out=dA, in_=dt, func=AF.Exp,
                                         scale=consts[:, AL + s:AL + s + 1])
                    if FAST:
                        # B broadcast in bf16 on PE
                        pb = ps_b.tile([D, L], F32, tag="bb")
                        lhsb = constsb[:, s:s + 1].to_broadcast([D, D])
                        for c in range(2):
                            nc.tensor.matmul(
                                pb[:, c * HALF:(c + 1) * HALF], lhsb,
                                xc_bf[:, c * HALF:(c + 1) * HALF],
                                start=True, stop=True)
                        # stage B to SBUF bf16 on the (slack) Scalar engine
                        b_sb = innerp.tile([D, L], BF, tag="bsb")
                        nc.scalar.copy(out=b_sb, in_=pb)
                        # u = dtx * B (bf16); every other iter on GpSimd to
                        # offload the DVE — deep bufs keep the scan fed
                        u = innerp.tile([D, L], BF, tag="u")
                        ueng = nc.gpsimd if s % 2 == 0 else nc.vector
                        ueng.tensor_tensor(out=u, in0=dtx_bf, in1=b_sb,
                                           op=OP.mult)
                        # h = scan(dA fp32, u bf16) -> h bf16
                        h = innerp.tile([D, L], BF, tag="h")
                        nc.vector.tensor_tensor_scan(
                            out=h, data0=dA, data1=u, initial=0.0,
                            op0=OP.mult, op1=OP.add)
                        # hG = h * G on GpSimd (bf16 out: halves SBUF traffic)
                        hg = innerp.tile([D, L], BF, tag="hg")
                        nc.gpsimd.tensor_tensor(out=hg, in0=h, in1=G_bf,
                                                op=OP.mult)
                        # C broadcast in bf16 on PE
                        pc = ps_c.tile([D, L], F32, tag="cb")
                        lhsc = constsb[:, S + s:S + s + 1].to_broadcast([D, D])
                        for c in range(2):
                            nc.tensor.matmul(
                                pc[:, c * HALF:(c + 1) * HALF], lhsc,
                                xc_bf[:, c * HALF:(c + 1) * HALF],
                                start=True, stop=True)
                        # P_s = sum_t (hg/L) * C — deferred one iteration so
                        # the DVE has work while GpSimd computes hg
                        flush_pending()
                        pending = (hg, pc, s)
                        continue

                    # fp32 baseline path
                    pb = ps_b.tile([D, L], F32, tag="bb")
                    lhsb = consts[:, XP + R + s:XP + R + s + 1].to_broadcast([D, D])
                    for c in range(2):
                        nc.tensor.matmul(
                            pb[:, c * HALF:(c + 1) * HALF], lhsb,
                            xc[:, c * HALF:(c + 1) * HALF],
                            start=True, stop=True)
                    u = innerp.tile([D, L], F32, tag="u")
                    nc.vector.tensor_tensor(out=u, in0=dtx, in1=pb, op=OP.mult)
                    # h = scan(dA, u)
                    h = innerp.tile([D, L], F32, tag="h")
                    nc.vector.tensor_tensor_scan(
                        out=h, data0=dA, data1=u, initial=0.0,
                        op0=OP.mult, op1=OP.add)
                    # C broadcast
                    pc = ps_c.tile([D, L], F32, tag="cb")
                    lhsc = consts[:, XP + R + S + s:XP + R + S + s + 1] \
                        .to_broadcast([D, D])
                    for c in range(2):
                        nc.tensor.matmul(
                            pc[:, c * HALF:(c + 1) * HALF], lhsc,
                            xc[:, c * HALF:(c + 1) * HALF],
                            start=True, stop=True)
                    gc = innerp.tile([D, L], F32, tag="gc")
                    nc.vector.tensor_tensor(out=gc, in0=G, in1=pc, op=OP.mult)
                    # P_s = sum_t (h/L) * gc
                    nc.vector.scalar_tensor_tensor(
                        out=junk, in0=h, scalar=1.0 / L, in1=gc,
                        op0=OP.mult, op1=OP.mult,
                        accum_out=P_all[:, s:s + 1])

                flush_pending()
                # P_mat[:, n] = sum over cols 0..64
                nc.vector.tensor_reduce(
                    out=P_mat[:, n:n + 1], in_=P_all[:, 0:S + 1],
                    axis=mybir.AxisListType.X, op=OP.add)

            # out [nseq, DOUT] = P_mat.T @ W2T + out_b
            psum_out = ps_pre.tile([nseq, DOUT], F32, tag="big")
            nc.tensor.matmul(psum_out, P_mat, consts[:, W2L:W2L + DOUT],
                             start=True, stop=True)
            out_sb = rhsp.tile([nseq, DOUT], F32, tag="outsb")
            nc.vector.tensor_tensor(out=out_sb, in0=psum_out,
                                    in1=consts[0:nseq, OBL:OBL + DOUT],
                                    op=OP.add)
            nc.sync.dma_start(out=out_d.ap(), in_=out_sb)

    nc.compile()
    return nc


def _host_prep(x_exo, in_w, in_b, m_in_w, conv_w, conv_b, xproj_w, dt_w, dt_b,
               A_log, D_in, m_out_w, out_w, out_b):
    f32 = np.float32
    w1 = (m_in_w @ in_w[:, 0]).astype(f32)
    b1 = (m_in_w @ in_b).astype(f32)
    w1x, w1z = w1[:D], w1[D:]
    b1x, b1z = b1[:D], b1[D:]
    cw0 = conv_w[:, 0, 0]
    cw1 = conv_w[:, 0, 1]
    a0 = (cw0 * w1x).astype(f32)
    a1 = (cw1 * w1x).astype(f32)
    c0 = (cw0 * b1x).astype(f32)
    cb = (cw1 * b1x + conv_b).astype(f32)
    A = (-np.exp(A_log)).astype(f32)
    W2 = (out_w @ m_out_w).astype(f32)

    consts = np.zeros((D, NCOL), f32)
    consts[:, AL:AL + S] = A
    consts[:, XP:XP + R + 2 * S] = xproj_w.T
    consts[0:R, DTW:DTW + D] = dt_w.T
    consts[0:4, LXC:LXC + D] = np.stack([a1, cb, a0, c0])
    consts[0:2, LZ:LZ + D] = np.stack([w1z, b1z])
    consts[:, W2L:W2L + DOUT] = W2.T
    consts[:, DTB] = dt_b
    consts[:, DCL] = D_in
    consts[0:NSEQ, OBL:OBL + DOUT] = np.tile(out_b, (NSEQ, 1))

    import ml_dtypes
    constsb = np.ascontiguousarray(
        xproj_w.T[:, R:R + 2 * S]).astype(ml_dtypes.bfloat16)

    xv_all = np.ascontiguousarray(
        x_exo.transpose(0, 2, 1).reshape(B * V, L)).astype(f32)
    xv4 = np.zeros((B * V, 4, L), f32)
    xv4[:, 0] = xv_all                    # xv
    xv4[:, 1] = 1.0                       # ones
    xv4[:, 2, 1:] = xv_all[:, :-1]        # shift(xv)
    xv4[:, 3, 1:] = 1.0                   # i0 = (l > 0)
    return consts, constsb, xv4


def kernel(**inputs):
    consts, constsb, xv4 = _host_prep(
        inputs["x_exo"], inputs["in_w"], inputs["in_b"], inputs["m_in_w"],
        inputs["conv_w"], inputs["conv_b"], inputs["xproj_w"], inputs["dt_w"],
        inputs["dt_b"], inputs["A_log"], inputs["D"], inputs["m_out_w"],
        inputs["out_w"], inputs["out_b"])

    global LAST_EXEC_NS, LAST_RESULTS
    if _CACHE.get("nseq") != NSEQ:
        _CACHE["nc"] = _build(NSEQ)
        _CACHE["nseq"] = NSEQ
    nc = _CACHE["nc"]

    in_maps = []
    for c in range(NCORES):
        in_maps.append({
            "consts": consts,
            "constsb": constsb,
            "xv4": np.ascontiguousarray(xv4[c * NSEQ:(c + 1) * NSEQ]),
        })
    res = run_bass_kernel_spmd(nc, in_maps, core_ids=list(range(NCORES)),
                               trace=PROFILE)
    LAST_EXEC_NS = res.exec_time_ns
    LAST_RESULTS = res
    out = np.concatenate([res.results[c]["out"] for c in range(NCORES)], axis=0)
    if out.shape[0] == B * V:
        out = out.reshape(B, V, DOUT)
    return out.astype(np.float32)
